# revision 1
# baseline (speedup 1.0000x reference)
"""Multi-head attention, tensor-parallel over heads x data-parallel over batch.

8 NeuronCores: core c handles batch b=c//2, head-group g=c%2 (8 heads, 512 chans).
Each core computes its head-group's attention + partial output projection;
the two partials per batch are summed on the host (row-parallel Wo unshard).

Per-core dataflow (all matmuls fp32r):
  qT/kT = W.T @ xT        [chan, tok] transposed projections
  v     = xkv @ Wv        [tok, chan] natural projection, stored per head
                          pair as [v_even(64) | ones(64) | v_odd(64)]
  scoresT[j,i] = k.q      row-group packed pairs -> one [128,1024] psum
  expT  = exp(s*scale + maskbias)   one ScalarE op per (pair, ih, jb)
  AV:   lhsT = [v_h|ones] (M=128) -> po rows = [o_h | sums replicated]
  norm: rb = exp(-ln(sums)) on ScalarE, oT = po * rb on VectorE
  out   = oT.T @ Wo       partial output projection
"""

import functools

import numpy as np

import concourse.bacc as bacc
import concourse.hw_specs as hw_specs
import concourse.mybir as mybir
import concourse.tile as tile
from concourse import bass_utils

# Route Exp AND Ln to the combined "natural_log_exp_and_others" ACT table set.
# The default chooser picks the first set containing each function, which
# alternates exp-only / ln-only sets and costs a ~1.3us ACT_TABLE_LOAD per
# switch (33 loads, ~42us, measured). Set ids stay the true act_info.json
# indices - only the choice among sets is narrowed.
_ORIG_GAT = hw_specs.get_activation_tables


@functools.cache
def _gat_prefer_combined(module_arch):
    tabs = {k: set(v) for k, v in _ORIG_GAT(module_arch).items()}
    comb = "natural_log_exp_and_others"
    if comb in tabs and {mybir.ActivationFunctionType.Exp,
                         mybir.ActivationFunctionType.Ln} <= tabs[comb]:
        for name, fns in tabs.items():
            if name != comb:
                fns.discard(mybir.ActivationFunctionType.Exp)
                fns.discard(mybir.ActivationFunctionType.Ln)
    return tabs


hw_specs.get_activation_tables = _gat_prefer_combined
bacc.get_activation_tables = _gat_prefer_combined

B = 4
T = 1024          # tokens (N = L)
D = 1024          # model dim
CH = 64           # channels per head
G = 512           # channels per head-group (8 heads)
HPC = 8           # heads per core
SCALE = CH ** -0.5
NEG = -30000.0    # mask bias (exp(x + NEG) == 0)
F32 = mybir.dt.float32
R32 = mybir.dt.float32r

N_CORES = 8
KB = 8            # 128-row contraction blocks over D
TB = 8            # 128-token blocks
PAIRS = 4         # head pairs per core
VW = 192          # v-tile columns per head pair: [v_even | ones | v_odd]
AV_LAG = 3        # software-pipeline depth: AV trails scores/exp by this many jb

LAST_RESULTS = None
_CACHE = {}


def _emit(tc):
    nc = tc.nc
    xqT = nc.dram_tensor("xqT", [D, T], R32, kind="ExternalInput").ap()
    xkvT = nc.dram_tensor("xkvT", [D, T], R32, kind="ExternalInput").ap()
    wq = nc.dram_tensor("wq", [D, G], R32, kind="ExternalInput").ap()
    wk = nc.dram_tensor("wk", [D, G], R32, kind="ExternalInput").ap()
    wv = nc.dram_tensor("wv", [D, G], R32, kind="ExternalInput").ap()
    wo = nc.dram_tensor("wo", [G, D], R32, kind="ExternalInput").ap()
    mb = nc.dram_tensor("mb", [128, TB], F32, kind="ExternalInput").ap()
    vones = nc.dram_tensor("vones", [128, PAIRS * CH], R32,
                           kind="ExternalInput").ap()
    out = nc.dram_tensor("out", [T, D], F32, kind="ExternalOutput").ap()

    Exp = mybir.ActivationFunctionType.Exp
    Log = mybir.ActivationFunctionType.Ln

    with (
        tc.tile_pool(name="wpool", bufs=1) as wpool,
        tc.tile_pool(name="xpool", bufs=1) as xpool,
        tc.tile_pool(name="actpool", bufs=1) as actpool,
        tc.tile_pool(name="opool", bufs=2) as opool,
        tc.tile_pool(name="psum", bufs=1, space="PSUM") as psum,
    ):
        # ---------------- input DMA (ordered by first use) ----------------
        mask_t = wpool.tile([128, TB], F32, name="mask_t", tag="mask")
        nc.sync.dma_start(mask_t[:], mb[:])

        wv_t, xkv_t = [], []
        for k in range(KB):
            t5 = wpool.tile([128, G], R32, name=f"wv{k}", tag=f"wv{k}")
            nc.sync.dma_start(t5[:], wv[k * 128:(k + 1) * 128, :])
            wv_t.append(t5)
            t4 = xpool.tile([128, T], R32, name=f"xkv{k}", tag=f"xkv{k}")
            nc.sync.dma_start(t4[:], xkvT[k * 128:(k + 1) * 128, :])
            xkv_t.append(t4)

        # v tiles: the shared ones blocks land via DMA into the pair layout
        v_t = [actpool.tile([128, PAIRS * VW], R32, name=f"v{tb}",
                            tag=f"v{tb}") for tb in range(TB)]
        for tb in range(TB):
            v3 = v_t[tb].rearrange("p (pb c) -> p pb c", c=VW)
            nc.sync.dma_start(v3[:, :, CH:2 * CH],
                              vones.rearrange("p (pb c) -> p pb c", c=CH))

        wq_t, wk_t, xq_t = [], [], []
        for k in range(KB):
            t1 = wpool.tile([128, G], R32, name=f"wq{k}", tag=f"wq{k}")
            nc.sync.dma_start(t1[:], wq[k * 128:(k + 1) * 128, :])
            wq_t.append(t1)
            t2 = wpool.tile([128, G], R32, name=f"wk{k}", tag=f"wk{k}")
            nc.sync.dma_start(t2[:], wk[k * 128:(k + 1) * 128, :])
            wk_t.append(t2)
            t3 = xpool.tile([128, T], R32, name=f"xq{k}", tag=f"xq{k}")
            nc.sync.dma_start(t3[:], xqT[k * 128:(k + 1) * 128, :])
            xq_t.append(t3)
        # wo arrives late, into the xq slots that die after the q projections
        wo_t = []
        for m in range(PAIRS):
            t6 = xpool.tile([128, D], R32, name=f"wo{m}", tag=f"xq{4 + m}")
            nc.sync.dma_start(t6[:], wo[m * 128:(m + 1) * 128, :])
            wo_t.append(t6)

        # ---------------- v = xkv @ Wv (natural layout) ----------------
        for tb in range(TB):
            ps = psum.tile([128, 512], F32, name="ps_acc", tag="acc", bufs=2)
            for k in range(KB):
                nc.tensor.matmul(
                    ps[:],
                    xkv_t[k][:, tb * 128:(tb + 1) * 128],
                    wv_t[k][:],
                    start=(k == 0),
                    stop=(k == KB - 1),
                )
            v3 = v_t[tb].rearrange("p (pb three c) -> p pb three c",
                                   three=3, c=CH)
            ps3 = ps.rearrange("p (pb two c) -> p pb two c", two=2, c=CH)
            nc.vector.tensor_copy(v3[:, :, 0:1, :], ps3[:, :, 0:1, :])
            nc.vector.tensor_copy(v3[:, :, 2:3, :], ps3[:, :, 1:2, :])

        # ---------------- qT/kT projections (all pairs) ----------------
        qT_t = [actpool.tile([128, T], R32, name=f"qT{m}", tag=f"qT{m}")
                for m in range(PAIRS)]
        kT_t = [actpool.tile([128, T], R32, name=f"kT{m}", tag=f"kT{m}")
                for m in range(PAIRS)]
        for p in range(PAIRS):
            for src, w_t, dst in ((xq_t, wq_t, qT_t), (xkv_t, wk_t, kT_t)):
                for ic in range(2):
                    csl = slice(ic * 512, (ic + 1) * 512)
                    ps = psum.tile([128, 512], F32, name="ps_acc", tag="acc",
                                   bufs=2)
                    for k in range(KB):
                        nc.tensor.matmul(
                            ps[:],
                            w_t[k][:, p * 128:(p + 1) * 128],
                            src[k][:, csl],
                            start=(k == 0),
                            stop=(k == KB - 1),
                        )
                    nc.vector.tensor_copy(dst[p][:, csl], ps[:])

        # ---------------- attention ----------------
        # oT reuses the xq slots (dead: all q projections done above)
        oT_t = [xpool.tile([128, T], R32, name=f"oT{m}", tag=f"xq{m}")
                for m in range(PAIRS)]
        for p in range(PAIRS):
            oT = oT_t[p]
            for ih in range(2):
                isl = slice(ih * 512, (ih + 1) * 512)
                po = psum.tile([128, 1024], F32, name="po", tag="po", bufs=1)
                pend = []
                for jb in range(TB):
                    jsl = slice(jb * 128, (jb + 1) * 128)
                    # one [128, 1024] psum tile: bank0 = head 2p, bank1 = 2p+1
                    pss = psum.tile([128, 1024], F32, name="ps_s", tag="sc",
                                    bufs=2)
                    for h in (0, 1):
                        hsl = slice(h * 64, (h + 1) * 64)
                        nc.tensor.matmul(
                            pss[:, h * 512:(h + 1) * 512],
                            kT_t[p][hsl, jsl],
                            qT_t[p][hsl, isl],
                        )
                    # one exp for both heads; mask bias is per-partition (= j)
                    et = wpool.tile([128, 1024], R32, name="et",
                                    tag=f"wv{jb % 4}")
                    nc.scalar.activation(et[:], pss[:], Exp,
                                         bias=mask_t[:, jb:jb + 1],
                                         scale=SCALE)
                    pend.append((jb, et))
                    if len(pend) > AV_LAG:
                        _av(nc, pend.pop(0), p, po, v_t)
                while pend:
                    _av(nc, pend.pop(0), p, po, v_t)
                # normalize. po bank0 = [o_even | s_even], bank1 = [s_odd | o_odd]
                # rb = exp(-ln(s)) = 1/s on ScalarE (both fns in one ACT set)
                ln_e = wpool.tile([CH, 512], F32, name="ln_e", tag="wv4")
                nc.scalar.activation(ln_e[:], po[CH:128, 0:512], Log)
                rb_e = wpool.tile([CH, 512], F32, name="rb_e", tag="wv5")
                nc.scalar.activation(rb_e[:], ln_e[:], Exp, scale=-1.0)
                nc.vector.tensor_mul(oT[0:CH, isl], po[0:CH, 0:512], rb_e[:])
                ln_o = wpool.tile([CH, 512], F32, name="ln_o", tag="wv6")
                nc.scalar.activation(ln_o[:], po[0:CH, 512:1024], Log)
                rb_o = wpool.tile([CH, 512], F32, name="rb_o", tag="wv7")
                nc.scalar.activation(rb_o[:], ln_o[:], Exp, scale=-1.0)
                nc.vector.tensor_mul(oT[CH:128, isl], po[CH:128, 512:1024],
                                     rb_o[:])

        # ---------------- out = oT.T @ Wo ----------------
        for tb in range(TB):
            tsl = slice(tb * 128, (tb + 1) * 128)
            for ncx in range(2):
                nsl = slice(ncx * 512, (ncx + 1) * 512)
                ps = psum.tile([128, 512], F32, name="ps_acc", tag="acc",
                               bufs=2)
                for m in range(PAIRS):
                    nc.tensor.matmul(
                        ps[:],
                        oT_t[m][:, tsl],
                        wo_t[m][:, nsl],
                        start=(m == 0),
                        stop=(m == PAIRS - 1),
                    )
                ot = opool.tile([128, 512], F32, name="ot", tag="ot")
                nc.vector.tensor_copy(ot[:], ps[:])
                nc.scalar.dma_start(out[tsl, nsl], ot[:])


def _av(nc, item, p, po, v_t):
    jb, et = item
    for h in (0, 1):
        # head 2p+h stationary: [v|ones] for h=0, [ones|v] for h=1
        csl = slice(p * VW + h * CH, p * VW + h * CH + 128)
        nc.tensor.matmul(
            po[:, h * 512:(h + 1) * 512],
            v_t[jb][:, csl],
            et[:, h * 512:(h + 1) * 512],
            start=(jb == 0),
            stop=(jb == TB - 1),
        )


def build_nc():
    nc = bacc.Bacc("TRN2", target_bir_lowering=False, debug=False,
                   num_devices=N_CORES)
    with tile.TileContext(nc) as tc:
        _emit(tc)
    nc.compile()
    return nc


def _get_compiled():
    if "nc" not in _CACHE:
        _CACHE["nc"] = build_nc()
    return _CACHE["nc"]


def make_in_maps(x_q, x_kv, pad_mask):
    ones = np.ones((128, PAIRS * CH), np.float32)
    in_maps = []
    for c in range(N_CORES):
        b, g = divmod(c, 2)
        gs = slice(g * G, (g + 1) * G)
        mbias = np.where(pad_mask[b], np.float32(NEG), np.float32(0.0))
        in_maps.append({
            "xqT": np.ascontiguousarray(x_q[b].T),
            "xkvT": np.ascontiguousarray(x_kv[b].T),
            "wq": _W["q"][:, gs].copy(),
            "wk": _W["k"][:, gs].copy(),
            "wv": _W["v"][:, gs].copy(),
            "wo": _W["o"][gs, :].copy(),
            "mb": np.ascontiguousarray(mbias.astype(np.float32).reshape(TB, 128).T),
            "vones": ones,
        })
    return in_maps


_W = {}


def kernel(x_q, x_kv, pad_mask, Wq, Wk, Wv, Wo, bo):
    global LAST_RESULTS
    x_q = np.asarray(x_q, dtype=np.float32)
    x_kv = np.asarray(x_kv, dtype=np.float32)
    pad_mask = np.asarray(pad_mask)
    _W["q"] = np.asarray(Wq, dtype=np.float32)
    _W["k"] = np.asarray(Wk, dtype=np.float32)
    _W["v"] = np.asarray(Wv, dtype=np.float32)
    _W["o"] = np.asarray(Wo, dtype=np.float32)
    bo = np.asarray(bo, dtype=np.float32)

    nc = _get_compiled()
    in_maps = make_in_maps(x_q, x_kv, pad_mask)
    res = bass_utils.run_bass_kernel_spmd(nc, in_maps, list(range(N_CORES)))
    LAST_RESULTS = res
    outp = np.zeros((B, T, D), np.float32)
    for b in range(B):
        outp[b] = res.results[2 * b]["out"] + res.results[2 * b + 1]["out"]
    outp += bo[None, None, :]
    return outp



# revision 6
# speedup vs baseline: 1.3202x; 1.3202x over previous
"""Multi-head attention, tensor-parallel over heads x data-parallel over batch.

8 NeuronCores: core c handles batch b=c//2, head-group g=c%2 (8 heads, 512 chans).
Each core computes its head-group's attention + partial output projection;
the two partials per batch are summed on the host (row-parallel Wo unshard).

v2 (all-bf16 matmuls, continuous-PE schedule):
  - every matmul operand is bf16 (halves DMA + LDWEIGHTS; PSUM stays fp32)
  - softmax 1/sum on VectorE via reciprocal_approx_fast (ScalarE runs ONLY
    Exp -> single ACT table set, and the norm no longer clogs the ACT FIFO)
  - qk projections for pair p+1 are interleaved into pair p's attention
    jb-loop so TensorE never idles behind ScalarE (keeps the PE HAM
    un-throttled at 2.4 GHz)
  - input DMA issue split across Sync and GpSimd queues

Per-core dataflow:
  qT/kT = W.T @ xT        [chan, tok] transposed projections
  v     = xkv @ Wv        [tok, chan] natural projection, stored per head
                          pair as [ones|v_even|ones|v_odd] (64 cols each)
  scoresT[j,i] = k.q      row-group packed pairs -> one [128,1024] psum
  expT  = exp(s*scale + maskbias)   one ScalarE op per (pair, ih, jb), bf16 out
  AV:   lhsT = [ones|v_h] (M=128) -> po rows = [sums replicated | o_h]
        (sums on partitions 0:64 because reciprocal_approx_fast silently
        returns garbage for partition-offset input APs)
  norm: rb = recip(sums) on VectorE, oT = po * rb on VectorE (bf16 out)
  out   = oT.T @ Wo       partial output projection
"""

import numpy as np

import concourse.bacc as bacc
import concourse.mybir as mybir
import concourse.tile as tile
from concourse import bass_utils

B = 4
T = 1024          # tokens (N = L)
D = 1024          # model dim
CH = 64           # channels per head
G = 512           # channels per head-group (8 heads)
SCALE = CH ** -0.5
NEG = -30000.0    # mask bias (exp(x + NEG) == 0)
F32 = mybir.dt.float32
BF16 = mybir.dt.bfloat16
BF_NP = mybir.dt.np(mybir.dt.bfloat16)

N_CORES = 8
KB = 8            # 128-row contraction blocks over D
TB = 8            # 128-token blocks
PAIRS = 4         # head pairs per core
VW = 256          # v-tile columns per head pair: [ones|v_even|ones|v_odd]
AV_LAG = 3        # software-pipeline depth: AV trails scores/exp by this many jb

LAST_RESULTS = None
_CACHE = {}


def _emit(tc):
    nc = tc.nc
    xqT = nc.dram_tensor("xqT", [D, T], BF16, kind="ExternalInput").ap()
    xkvT = nc.dram_tensor("xkvT", [D, T], BF16, kind="ExternalInput").ap()
    wq = nc.dram_tensor("wq", [D, G], BF16, kind="ExternalInput").ap()
    wk = nc.dram_tensor("wk", [D, G], BF16, kind="ExternalInput").ap()
    wv = nc.dram_tensor("wv", [D, G], BF16, kind="ExternalInput").ap()
    wo = nc.dram_tensor("wo", [G, D], BF16, kind="ExternalInput").ap()
    mb = nc.dram_tensor("mb", [128, TB], F32, kind="ExternalInput").ap()
    vones = nc.dram_tensor("vones", [128, PAIRS * CH], BF16,
                           kind="ExternalInput").ap()
    out = nc.dram_tensor("out", [T, D], F32, kind="ExternalOutput").ap()

    Exp = mybir.ActivationFunctionType.Exp

    with (
        tc.tile_pool(name="wpool", bufs=1) as wpool,
        tc.tile_pool(name="xpool", bufs=1) as xpool,
        tc.tile_pool(name="apool", bufs=1) as apool,
        tc.tile_pool(name="epool", bufs=1) as epool,
        tc.tile_pool(name="npool", bufs=1) as npool,
        tc.tile_pool(name="opool", bufs=2) as opool,
        tc.tile_pool(name="psum", bufs=1, space="PSUM") as psum,
    ):
        # ---------------- input DMA (ordered by first use) ----------------
        # sync queue: v-projection inputs; gpsimd queue: q/k/o inputs.
        mask_t = wpool.tile([128, TB], F32, name="mask_t", tag="mask")
        nc.sync.dma_start(mask_t[:], mb[:])

        wv_t, xkv_t = [], []
        for k in range(KB):
            t5 = wpool.tile([128, G], BF16, name=f"wv{k}", tag=f"wv{k}")
            nc.sync.dma_start(t5[:], wv[k * 128:(k + 1) * 128, :])
            wv_t.append(t5)
            t4 = xpool.tile([128, T], BF16, name=f"xkv{k}", tag=f"xkv{k}")
            nc.sync.dma_start(t4[:], xkvT[k * 128:(k + 1) * 128, :])
            xkv_t.append(t4)

        # v tiles: the shared ones blocks land via DMA into the pair layout
        v_t = [apool.tile([128, PAIRS * VW], BF16, name=f"v{tb}",
                          tag=f"v{tb}") for tb in range(TB)]
        for tb in range(TB):
            v4 = v_t[tb].rearrange("p (pb f c) -> p pb f c", f=4, c=CH)
            for f in (0, 2):
                nc.sync.dma_start(v4[:, :, f:f + 1, :],
                                  vones.rearrange("p (pb o c) -> p pb o c",
                                                  o=1, c=CH))

        wq_t, wk_t, xq_t = [], [], []
        for k in range(KB):
            t1 = wpool.tile([128, G], BF16, name=f"wq{k}", tag=f"wq{k}")
            nc.gpsimd.dma_start(t1[:], wq[k * 128:(k + 1) * 128, :])
            wq_t.append(t1)
            t2 = wpool.tile([128, G], BF16, name=f"wk{k}", tag=f"wk{k}")
            nc.gpsimd.dma_start(t2[:], wk[k * 128:(k + 1) * 128, :])
            wk_t.append(t2)
            t3 = xpool.tile([128, T], BF16, name=f"xq{k}", tag=f"xq{k}")
            nc.gpsimd.dma_start(t3[:], xqT[k * 128:(k + 1) * 128, :])
            xq_t.append(t3)
        wo_t = []
        for m in range(PAIRS):
            t6 = wpool.tile([128, D], BF16, name=f"wo{m}", tag=f"wo{m}")
            nc.gpsimd.dma_start(t6[:], wo[m * 128:(m + 1) * 128, :])
            wo_t.append(t6)

        # ---------------- v = xkv @ Wv (natural layout) ----------------
        for tb in range(TB):
            ps = psum.tile([128, 512], F32, name="ps_acc", tag="acc", bufs=2)
            for k in range(KB):
                nc.tensor.matmul(
                    ps[:],
                    xkv_t[k][:, tb * 128:(tb + 1) * 128],
                    wv_t[k][:],
                    start=(k == 0),
                    stop=(k == KB - 1),
                )
            v4 = v_t[tb].rearrange("p (pb four c) -> p pb four c",
                                   four=4, c=CH)
            ps3 = ps.rearrange("p (pb two c) -> p pb two c", two=2, c=CH)
            nc.vector.tensor_copy(v4[:, :, 1:2, :], ps3[:, :, 0:1, :])
            nc.vector.tensor_copy(v4[:, :, 3:4, :], ps3[:, :, 1:2, :])

        # ---------------- qT/kT projection chains ----------------
        qT_t = [apool.tile([128, T], BF16, name=f"qT{m}", tag=f"qT{m}")
                for m in range(PAIRS)]
        kT_t = [apool.tile([128, T], BF16, name=f"kT{m}", tag=f"kT{m}")
                for m in range(PAIRS)]

        def proj_chain(p, which, ic):
            src, w_t, dst = ((xq_t, wq_t, qT_t), (xkv_t, wk_t, kT_t))[which]
            csl = slice(ic * 512, (ic + 1) * 512)
            ps = psum.tile([128, 512], F32, name="ps_acc", tag="acc", bufs=2)
            for k in range(KB):
                nc.tensor.matmul(
                    ps[:],
                    w_t[k][:, p * 128:(p + 1) * 128],
                    src[k][:, csl],
                    start=(k == 0),
                    stop=(k == KB - 1),
                )
            nc.vector.tensor_copy(dst[p][:, csl], ps[:])

        for which in range(2):
            for ic in range(2):
                proj_chain(0, which, ic)

        # ---------------- attention ----------------
        oT_t = [apool.tile([128, T], BF16, name=f"oT{m}", tag=f"oT{m}")
                for m in range(PAIRS)]
        for p in range(PAIRS):
            # projection chains for the NEXT pair, interleaved into this
            # pair's jb loop to keep TensorE fed while ScalarE runs exps
            fill = ([(p + 1, w, ic) for w in range(2) for ic in range(2)]
                    if p + 1 < PAIRS else [])
            oT = oT_t[p]
            for ih in range(2):
                isl = slice(ih * 512, (ih + 1) * 512)
                po = psum.tile([128, 1024], F32, name="po", tag="po", bufs=1)
                pend = []
                for jb in range(TB):
                    jsl = slice(jb * 128, (jb + 1) * 128)
                    # one [128, 1024] psum tile: bank0 = head 2p, bank1 = 2p+1
                    pss = psum.tile([128, 1024], F32, name="ps_s", tag="sc",
                                    bufs=2)
                    for h in (0, 1):
                        hsl = slice(h * 64, (h + 1) * 64)
                        nc.tensor.matmul(
                            pss[:, h * 512:(h + 1) * 512],
                            kT_t[p][hsl, jsl],
                            qT_t[p][hsl, isl],
                        )
                    # one exp for both heads; mask bias is per-partition (= j)
                    et = epool.tile([128, 1024], BF16, name="et", tag="et",
                                    bufs=6)
                    nc.scalar.activation(et[:], pss[:], Exp,
                                         bias=mask_t[:, jb:jb + 1],
                                         scale=SCALE)
                    pend.append((jb, et))
                    if len(pend) > AV_LAG:
                        _av(nc, pend.pop(0), p, po, v_t)
                    if jb in (2, 5) and fill:
                        proj_chain(*fill.pop(0))
                while pend:
                    _av(nc, pend.pop(0), p, po, v_t)
                # normalize. po bank h = [s_h (64 rows) | o_h (64 rows)]
                # rb = 1/s on VectorE (~51 ULP approx; sums are O(1..1e3));
                # recip inputs MUST be partition-offset-0 APs
                rb_e = npool.tile([CH, 512], F32, name="rb_e", tag="rb",
                                  bufs=4)
                nc.vector.reciprocal_approx_fast(rb_e[:], po[0:CH, 0:512])
                nc.vector.tensor_mul(oT[0:CH, isl], po[CH:128, 0:512],
                                     rb_e[:])
                rb_o = npool.tile([CH, 512], F32, name="rb_o", tag="rb",
                                  bufs=4)
                nc.vector.reciprocal_approx_fast(rb_o[:], po[0:CH, 512:1024])
                nc.vector.tensor_mul(oT[CH:128, isl], po[CH:128, 512:1024],
                                     rb_o[:])

        # ---------------- out = oT.T @ Wo ----------------
        for tb in range(TB):
            tsl = slice(tb * 128, (tb + 1) * 128)
            for ncx in range(2):
                nsl = slice(ncx * 512, (ncx + 1) * 512)
                ps = psum.tile([128, 512], F32, name="ps_acc", tag="acc",
                               bufs=2)
                for m in range(PAIRS):
                    nc.tensor.matmul(
                        ps[:],
                        oT_t[m][:, tsl],
                        wo_t[m][:, nsl],
                        start=(m == 0),
                        stop=(m == PAIRS - 1),
                    )
                ot = opool.tile([128, 512], F32, name="ot", tag="ot")
                nc.vector.tensor_copy(ot[:], ps[:])
                nc.scalar.dma_start(out[tsl, nsl], ot[:])


def _av(nc, item, p, po, v_t):
    jb, et = item
    for h in (0, 1):
        # head 2p+h stationary: [ones|v_h] -> out rows [sums | o_h]
        csl = slice(p * VW + h * 128, p * VW + h * 128 + 128)
        nc.tensor.matmul(
            po[:, h * 512:(h + 1) * 512],
            v_t[jb][:, csl],
            et[:, h * 512:(h + 1) * 512],
            start=(jb == 0),
            stop=(jb == TB - 1),
        )


def build_nc():
    nc = bacc.Bacc("TRN2", target_bir_lowering=False, debug=False,
                   num_devices=N_CORES)
    with tile.TileContext(nc) as tc:
        _emit(tc)
    nc.compile()
    return nc


def _get_compiled():
    if "nc" not in _CACHE:
        _CACHE["nc"] = build_nc()
    return _CACHE["nc"]


def make_in_maps(x_q, x_kv, pad_mask):
    ones = np.ones((128, PAIRS * CH), BF_NP)
    in_maps = []
    for c in range(N_CORES):
        b, g = divmod(c, 2)
        gs = slice(g * G, (g + 1) * G)
        mbias = np.where(pad_mask[b], np.float32(NEG), np.float32(0.0))
        in_maps.append({
            "xqT": np.ascontiguousarray(x_q[b].T).astype(BF_NP),
            "xkvT": np.ascontiguousarray(x_kv[b].T).astype(BF_NP),
            "wq": _W["q"][:, gs].astype(BF_NP),
            "wk": _W["k"][:, gs].astype(BF_NP),
            "wv": _W["v"][:, gs].astype(BF_NP),
            "wo": np.ascontiguousarray(_W["o"][gs, :]).astype(BF_NP),
            "mb": np.ascontiguousarray(mbias.astype(np.float32).reshape(TB, 128).T),
            "vones": ones,
        })
    return in_maps


_W = {}


def kernel(x_q, x_kv, pad_mask, Wq, Wk, Wv, Wo, bo):
    global LAST_RESULTS
    x_q = np.asarray(x_q, dtype=np.float32)
    x_kv = np.asarray(x_kv, dtype=np.float32)
    pad_mask = np.asarray(pad_mask)
    _W["q"] = np.asarray(Wq, dtype=np.float32)
    _W["k"] = np.asarray(Wk, dtype=np.float32)
    _W["v"] = np.asarray(Wv, dtype=np.float32)
    _W["o"] = np.asarray(Wo, dtype=np.float32)
    bo = np.asarray(bo, dtype=np.float32)

    nc = _get_compiled()
    in_maps = make_in_maps(x_q, x_kv, pad_mask)
    res = bass_utils.run_bass_kernel_spmd(nc, in_maps, list(range(N_CORES)))
    LAST_RESULTS = res
    outp = np.zeros((B, T, D), np.float32)
    for b in range(B):
        outp[b] = res.results[2 * b]["out"] + res.results[2 * b + 1]["out"]
    outp += bo[None, None, :]
    return outp
